# revision 20
# baseline (speedup 1.0000x reference)
"""GAT (3-layer, 8-head) forward on 8 Trainium2 NeuronCores.

Strategy (graph/data parallel, per sharding hint):
  - Nodes are re-partitioned into 160 slots (8 cores x 20 blocks of 128 lanes)
    by LPT bin-packing on in-degree plus a within-core repair pass, so every
    block needs exactly 7 remote-gather chunks + 2 local chunks (self-loops
    and same-core-source edges, capped at 256 with spill to remote).
  - bf16 on the hot path: features, weights, the AllGather payload [h|e_src],
    the halo gather, and the 0/1 scatter-mask matmuls (4x TensorE rate, half
    the DMA/collective bytes).  PSUM accumulation stays fp32.
  - Scatter masks are generated on-chip (is_equal compare against an iota
    row); transposed masks (for the e_dst expansion matmul) are built once in
    layer 0 and kept resident in SBUF.
  - Local chunks are gathered from the core's own ag_in DURING the AllGather
    window (they do not depend on the collective), hiding their issue cost.
  - The per-layer work is software-pipelined front/back per block: front =
    gathers + mask compare + e_dst matmuls + leaky/exp + ee*h; back = the
    scatter matmuls + alpha-normalize + LayerNorm + relu + the NEXT layer's
    GEMM-A/attention-logit staging.  The LayerNorm uses DVE bn_stats/bn_aggr
    and a magic-seed Newton rsqrt so the scalar engine runs only Exp (one
    activation-table load total instead of two reloads per block).
  - Mean-pool via 0/1 pool-mask matmuls folded into the last layer's edge
    phase; the FC head runs per-core on the partial pool (FC is linear), so
    the final AllReduce is only [G, OUT]; exact 1/count scaling afterwards.
"""

import os
import sys

sys.path.insert(0, "/opt/trn_rl_repo")

import heapq

import numpy as np
import ml_dtypes

import concourse.bass as bass
import concourse.mybir as mybir
import concourse.tile as tile
from concourse import bacc
from concourse.bass_utils import run_bass_kernel_spmd
from concourse.masks import make_identity

F32 = mybir.dt.float32
BF16 = mybir.dt.bfloat16
I32 = mybir.dt.int32
ALU = mybir.AluOpType
ACT = mybir.ActivationFunctionType
BF = ml_dtypes.bfloat16

P = 128

LAST_RESULTS = None


def _full_cfg():
    return dict(
        n_cores=8,
        N=20000,
        D=512,
        H=8,
        G=128,
        OUT=128,
        NEG=0.2,
        EPS=1e-5,
        L=3,
    )


# --------------------------------------------------------------------------
# Host-side preprocessing: index manipulation / relayout / dtype casts only.
# --------------------------------------------------------------------------


def _prep(inputs, cfg):
    nc_ = cfg["n_cores"]
    N, D, H, G, OUT, L = cfg["N"], cfg["D"], cfg["H"], cfg["G"], cfg["OUT"], cfg["L"]
    C = D // H

    x = np.asarray(inputs["x"], np.float32)
    ei = np.asarray(inputs["edge_index"])
    batch = np.asarray(inputs["batch"]).astype(np.int64)

    SH = ((N // nc_ + P - 1) // P) * P  # padded rows per core
    NB = SH // P
    KD = D // P
    NSLOT = nc_ * NB
    SEG = 2
    SEGR = SH // SEG

    # --- LPT bin-packing of nodes into slots by non-self in-degree
    deg = np.bincount(ei[1], minlength=N)
    order = np.argsort(-deg, kind="stable")
    heap = [(0, 0, s) for s in range(NSLOT)]
    heapq.heapify(heap)
    slot_of = np.zeros(N, np.int32)
    lane_of = np.zeros(N, np.int32)
    for n in order:
        while True:
            e, cnt, s = heapq.heappop(heap)
            if cnt < P:
                break
        slot_of[n] = s
        lane_of[n] = cnt
        heapq.heappush(heap, (e + int(deg[n]), cnt + 1, s))

    core_of = slot_of // NB
    blk_of = slot_of % NB
    src, dst = ei[0], ei[1]

    # --- within-core repair: cap remote in-edges at 7x128 and local (incl
    # self-loop) at 2x128 per block by moving nodes between blocks
    rin = np.bincount(dst[core_of[src] != core_of[dst]], minlength=N)
    lin = np.bincount(dst[core_of[src] == core_of[dst]], minlength=N) + 1
    REM_CAP, LOC_CAP = 7 * P, 2 * P
    for c in range(nc_):
        nodes_c = np.nonzero(core_of == c)[0]
        blocks = [list(nodes_c[blk_of[nodes_c] == b]) for b in range(NB)]
        rem = np.array([rin[bl].sum() for bl in blocks], np.int64)
        loc = np.array([lin[bl].sum() for bl in blocks], np.int64)
        cnt = np.array([len(bl) for bl in blocks], np.int64)

        def eff(b):
            return rem[b] + max(0, loc[b] - LOC_CAP)

        for _ in range(2000):
            effs = [eff(b) for b in range(NB)]
            b_bad = int(np.argmax(effs))
            if effs[b_bad] <= REM_CAP:
                break
            cand = [b for b in range(NB) if cnt[b] < P and b != b_bad]
            b_tgt = min(cand, key=lambda b: eff(b))
            n_mv = max(blocks[b_bad], key=lambda n: rin[n])
            blocks[b_bad].remove(n_mv)
            blocks[b_tgt].append(n_mv)
            rem[b_bad] -= rin[n_mv]; rem[b_tgt] += rin[n_mv]
            loc[b_bad] -= lin[n_mv]; loc[b_tgt] += lin[n_mv]
            cnt[b_bad] -= 1; cnt[b_tgt] += 1
        for b in range(NB):
            for j, n in enumerate(blocks[b]):
                slot_of[n] = c * NB + b
                lane_of[n] = j
    core_of = slot_of // NB
    blk_of = slot_of % NB
    row_of = blk_of * P + lane_of  # core-local row
    pid = core_of * SH + row_of

    # --- group edges: per (core, block): remote chunks, then local chunks
    NR_CH, NL_CH = 7, 2
    CH = NR_CH + NL_CH
    gidx = np.zeros((nc_, P, NB, NR_CH), np.int32)
    lidx = np.zeros((nc_, P, NB, NL_CH), np.int32)
    dloc = np.full((nc_, P, NB, CH), 200.0, np.float32)
    for c in range(nc_):
        sel = np.nonzero(core_of[dst] == c)[0]
        for b in range(NB):
            m = sel[blk_of[dst[sel]] == b]
            is_loc = core_of[src[m]] == c
            # local pool: self-loops + local-src edges (cap 2*128, spill->remote)
            slot_nodes = np.nonzero(slot_of == c * NB + b)[0]
            lp_src = np.concatenate([slot_nodes, src[m[is_loc]]])
            lp_dl = np.concatenate([lane_of[slot_nodes], lane_of[dst[m[is_loc]]]])
            spill = len(lp_src) - LOC_CAP
            if spill > 0:
                sp_src, sp_dl = lp_src[LOC_CAP:], lp_dl[LOC_CAP:]
                lp_src, lp_dl = lp_src[:LOC_CAP], lp_dl[:LOC_CAP]
            else:
                sp_src = np.zeros(0, np.int64); sp_dl = np.zeros(0, np.int64)
            rp_src = np.concatenate([src[m[~is_loc]], sp_src])
            rp_dl = np.concatenate([lane_of[dst[m[~is_loc]]], sp_dl])
            assert len(rp_src) <= NR_CH * P, (c, b, len(rp_src))
            j = np.arange(len(rp_src))
            gidx[c, j % P, b, j // P] = pid[rp_src]
            dloc[c, j % P, b, j // P] = rp_dl
            j = np.arange(len(lp_src))
            lidx[c, j % P, b, j // P] = row_of[lp_src]
            dloc[c, j % P, b, NR_CH + j // P] = lp_dl

    # --- x shards in slot order, feature-major (KD, 128, SH), bf16
    xT = np.zeros((nc_, KD, P, SH), BF)
    for c in range(nc_):
        xp = np.zeros((SH, D), np.float32)
        nodes = np.nonzero(core_of == c)[0]
        xp[row_of[nodes]] = x[nodes]
        xT[c] = xp.T.reshape(KD, P, SH).astype(BF)

    # --- weights (bf16)
    W_all = np.zeros((L, KD, P, D), BF)
    WT_all = np.zeros((L, KD, P, D), BF)
    A_all = np.zeros((L, KD, P, 2 * H), BF)
    b_l, g_l, be_l = [], [], []
    for l in range(L):
        W = np.asarray(inputs[f"W{l}"], np.float32)
        W_all[l] = W.reshape(KD, P, D).astype(BF)
        WT_all[l] = np.ascontiguousarray(W.T).reshape(KD, P, D).astype(BF)
        A = np.zeros((D, 2 * H), np.float32)
        a_s = np.asarray(inputs[f"as{l}"], np.float32)
        a_d = np.asarray(inputs[f"ad{l}"], np.float32)
        for h in range(H):
            A[h * C : (h + 1) * C, h] = a_s[h]
            A[h * C : (h + 1) * C, H + h] = a_d[h]
        A_all[l] = A.reshape(KD, P, 2 * H).astype(BF)
        b_l.append(np.asarray(inputs[f"b{l}"], np.float32))
        g_l.append(np.asarray(inputs[f"g{l}"], np.float32))
        be_l.append(np.asarray(inputs[f"be{l}"], np.float32))

    skip_b = all(not b.any() for b in b_l)
    skip_g = all((g == 1.0).all() for g in g_l)
    skip_be = all(not be.any() for be in be_l)

    fc_W = np.asarray(inputs["fc_W"], np.float32).reshape(KD, P, OUT)
    fc_b = np.asarray(inputs["fc_b"], np.float32)
    skip_fcb = not fc_b.any()

    # --- pool masks (0/1) in slot order; exact 1/count applied in fp32 tail
    cnt_g = np.bincount(batch, minlength=G).astype(np.float64)
    inv_cnt = (1.0 / np.maximum(cnt_g, 1.0)).astype(np.float32)[:, None]
    poolmask = np.zeros((nc_, P, NB, G), BF)
    for c in range(nc_):
        nodes = np.nonzero(core_of == c)[0]
        poolmask[c, lane_of[nodes], blk_of[nodes], batch[nodes]] = 1.0

    iota = np.broadcast_to(np.arange(P, dtype=np.float32), (P, P)).copy().astype(BF)

    meta = dict(
        SH=SH, NB=NB, KD=KD, CH=CH, NR_CH=NR_CH, NL_CH=NL_CH, ROW=D + H,
        skip_b=skip_b, skip_g=skip_g, skip_be=skip_be, skip_fcb=skip_fcb,
    )

    in_maps = []
    for c in range(nc_):
        m = dict(
            xT=xT[c],
            W_all=W_all,
            WT_all=WT_all,
            A_all=A_all,
            fc_W=fc_W,
            gidx=gidx[c],
            lidx=lidx[c],
            dloc=dloc[c].astype(BF),
            iota=iota,
            poolmask=poolmask[c],
            invcnt=inv_cnt,
        )
        if not skip_b:
            m["b_rep"] = np.broadcast_to(
                np.stack(b_l)[:, None, :], (L, P, D)
            ).copy()
        if not skip_g:
            m["g_rep"] = np.broadcast_to(
                np.stack(g_l)[:, None, :], (L, P, D)
            ).copy()
        if not skip_be:
            m["be_rep"] = np.broadcast_to(
                np.stack(be_l)[:, None, :], (L, P, D)
            ).copy()
        if not skip_fcb:
            m["fcb_rep"] = np.broadcast_to(fc_b[None, :], (P, OUT)).copy()
        in_maps.append(m)
    return in_maps, meta


# --------------------------------------------------------------------------
# Device program
# --------------------------------------------------------------------------


def build(tc, cfg, meta, I, out_ap):
    nc = tc.nc
    nc_cores = cfg["n_cores"]
    D, H, G, OUT, L = cfg["D"], cfg["H"], cfg["G"], cfg["OUT"], cfg["L"]
    NEG, EPS = cfg["NEG"], cfg["EPS"]
    SH, NB, KD, ROW = meta["SH"], meta["NB"], meta["KD"], meta["ROW"]
    CH, NR_CH, NL_CH = meta["CH"], meta["NR_CH"], meta["NL_CH"]
    H2 = 2 * H

    rg = [list(range(nc_cores))]
    shared = "Shared" if nc_cores > 4 else "Local"

    from contextlib import ExitStack

    ctx = ExitStack()
    res = ctx.enter_context(tc.tile_pool(name="res", bufs=1))
    dram = ctx.enter_context(tc.tile_pool(name="dram", bufs=1, space="DRAM"))
    psum = ctx.enter_context(tc.tile_pool(name="psum", bufs=1, space="PSUM"))
    sb = ctx.enter_context(tc.tile_pool(name="sb", bufs=1))

    # ---------------- resident tiles
    xT_sb = [res.tile([P, SH], BF16, name=f"xT{k}") for k in range(KD)]
    W_sb = [[res.tile([P, D], BF16, name=f"W{l}_{k}") for k in range(KD)]
            for l in range(L)]
    wa_sb = [[res.tile([P, H2], BF16, name=f"wa{l}_{k}") for k in range(KD)]
             for l in range(L)]
    henm_sb = [res.tile([P, NB, H2], BF16, name=f"henm{i}") for i in range(2)]
    mkT_sb = res.tile([P, NB, CH, P], BF16, name="mkT")
    gidx_sb = res.tile([P, NB, NR_CH], I32, name="gidx")
    lidx_sb = res.tile([P, NB, NL_CH], I32, name="lidx")
    gtl_sb = res.tile([P, NB, NL_CH, ROW], BF16, name="gtl")
    dloc_sb = res.tile([P, NB, CH], BF16, name="dloc")
    iota_sb = res.tile([P, P], BF16, name="iota")
    pm_sb = res.tile([P, NB, G], BF16, name="pm")
    id128b = res.tile([P, P], BF16, name="id128b")
    idh2b = res.tile([H2, H2], BF16, name="idh2b")
    make_identity(nc, id128b[:])
    make_identity(nc, idh2b[:])

    b_rep = g_rep = be_rep = None
    if not meta["skip_b"]:
        b_rep = [res.tile([P, D], F32, name=f"b_rep{l}") for l in range(L)]
    if not meta["skip_g"]:
        g_rep = [res.tile([P, D], F32, name=f"g_rep{l}") for l in range(L)]
    if not meta["skip_be"]:
        be_rep = [res.tile([P, D], F32, name=f"be_rep{l}") for l in range(L)]

    nc.sync.dma_start(out=gidx_sb[:], in_=I["gidx"][:])
    nc.sync.dma_start(out=lidx_sb[:], in_=I["lidx"][:])
    nc.sync.dma_start(out=dloc_sb[:], in_=I["dloc"][:])
    nc.sync.dma_start(out=iota_sb[:], in_=I["iota"][:])
    nc.sync.dma_start(out=pm_sb[:], in_=I["poolmask"][:])
    for k in range(KD):
        nc.sync.dma_start(out=xT_sb[k][:], in_=I["xT"][k])
    for l in range(L):
        for k in range(KD):
            nc.sync.dma_start(out=W_sb[l][k][:], in_=I["W_all"][l, k])
        if b_rep is not None:
            nc.sync.dma_start(out=b_rep[l][:], in_=I["b_rep"][l])
        if g_rep is not None:
            nc.sync.dma_start(out=g_rep[l][:], in_=I["g_rep"][l])
        if be_rep is not None:
            nc.sync.dma_start(out=be_rep[l][:], in_=I["be_rep"][l])

    # ---------------- DRAM comm buffers (bf16)
    ag_ins = [dram.tile([SH, ROW], BF16, name=f"ag_in{l}") for l in range(L)]
    ag_outs = [
        dram.tile([nc_cores * SH, ROW], BF16, name=f"ag_out{l}", addr_space=shared)
        for l in range(L)
    ]
    ar_in = dram.tile([G, OUT], F32, name="ar_in")
    ar_out = dram.tile([G, OUT], F32, name="ar_out", addr_space=shared)

    def allgather(l):
        nc.gpsimd.collective_compute(
            "AllGather", ALU.bypass, replica_groups=rg,
            ins=[ag_ins[l].opt()], outs=[ag_outs[l].opt()],
        )
        # local-source chunks gathered from ag_in while the collective runs
        for b in range(NB):
            for ch in range(NL_CH):
                nc.gpsimd.indirect_dma_start(
                    out=gtl_sb[:, b, ch, :],
                    out_offset=None,
                    in_=ag_ins[l][:],
                    in_offset=bass.IndirectOffsetOnAxis(
                        ap=lidx_sb[:, b, ch : ch + 1], axis=0
                    ),
                )

    # ---------------- WA = W @ A for all layers (transient WT/A pool)
    with tc.tile_pool(name="wt_pool", bufs=1) as wtp:
        for l in range(L):
            WT_t = [wtp.tile([P, D], BF16, name=f"WT{k}", tag=f"wt{k}", bufs=1)
                    for k in range(KD)]
            A_t = [wtp.tile([P, H2], BF16, name=f"A{k}", tag=f"a{k}", bufs=1)
                   for k in range(KD)]
            for k in range(KD):
                nc.sync.dma_start(out=WT_t[k][:], in_=I["WT_all"][l, k])
                nc.sync.dma_start(out=A_t[k][:], in_=I["A_all"][l, k])
            for ic in range(KD):
                wa_ps = psum.tile([P, H2], F32, name="wa_ps", tag="ed", bufs=2)
                for oc in range(KD):
                    nc.tensor.matmul(
                        out=wa_ps[:],
                        lhsT=WT_t[oc][:, ic * P : (ic + 1) * P],
                        rhs=A_t[oc][:],
                        start=(oc == 0),
                        stop=(oc == KD - 1),
                    )
                nc.vector.tensor_copy(out=wa_sb[l][ic][:], in_=wa_ps[:])

    # ---------------- helper: GEMM-A + attention logits for (layer, block)
    def gemm_he_stage(l, b):
        bs = slice(b * P, (b + 1) * P)
        h_ps = psum.tile([P, D], F32, name="h_ps", tag="big", bufs=2)
        for k in range(KD):
            nc.tensor.matmul(
                out=h_ps[:],
                lhsT=xT_sb[k][:, bs],
                rhs=W_sb[l][k][:],
                start=(k == 0),
                stop=(k == KD - 1),
            )
        he_ps = psum.tile([H2, P], F32, name="he_ps", tag="ed", bufs=2)
        for k in range(KD):
            nc.tensor.matmul(
                out=he_ps[:],
                lhsT=wa_sb[l][k][:],
                rhs=xT_sb[k][:, bs],
                start=(k == 0),
                stop=(k == KD - 1),
            )
        he_sb = sb.tile([H2, P], BF16, name="he_sb", tag="he_sb", bufs=2)
        nc.scalar.activation(he_sb[:], he_ps[:], ACT.Identity)
        henm_ps = psum.tile([P, H2], BF16, name="henm_ps", tag="ed", bufs=2)
        nc.tensor.transpose(out=henm_ps[:], in_=he_sb[:], identity=idh2b[:])
        nc.scalar.activation(henm_sb[l % 2][:, b, :], henm_ps[:], ACT.Identity)
        stage = sb.tile([P, ROW], BF16, name="stage", tag="stage", bufs=3)
        nc.scalar.activation(stage[:, 0:D], h_ps[:], ACT.Identity)
        nc.scalar.activation(
            stage[:, D:ROW], henm_sb[l % 2][:, b, 0:H], ACT.Identity
        )
        nc.sync.dma_start(out=ag_ins[l][bs, :], in_=stage[:])

    # ---------------- layer-0 prologue (segmented AG0)
    for b in range(NB):
        gemm_he_stage(0, b)
    allgather(0)

    # ---------------- layers (front/back software pipeline)
    pool_ps = psum.tile([G, D], F32, name="pool_ps", tag="pool", bufs=1)

    MAGIC = 0x5F3759DF

    def front(l, b):
        henm_cur = henm_sb[l % 2]
        bs = slice(b * P, (b + 1) * P)
        gt = sb.tile([P, CH, ROW], BF16, name="gt", tag="gt", bufs=3)
        for ch in range(NR_CH):
            nc.gpsimd.indirect_dma_start(
                out=gt[:, ch, :],
                out_offset=None,
                in_=ag_outs[l][:],
                in_offset=bass.IndirectOffsetOnAxis(
                    ap=gidx_sb[:, b, ch : ch + 1], axis=0
                ),
            )
        for ch in range(NL_CH):
            nc.scalar.activation(
                gt[:, NR_CH + ch, :], gtl_sb[:, b, ch, :], ACT.Identity
            )
        mk_sb = sb.tile([P, CH, P], BF16, name="mk_sb", tag="mk", bufs=3)
        nc.vector.tensor_tensor(
            out=mk_sb[:],
            in0=dloc_sb[:, b, :].unsqueeze(2).to_broadcast([P, CH, P]),
            in1=iota_sb[:].unsqueeze(1).to_broadcast([P, CH, P]),
            op=ALU.is_equal,
        )
        ed_ps = psum.tile([P, CH, H], F32, name="ed_ps", tag="ed", bufs=2)
        for ch in range(CH):
            if l == 0:
                mkT_ps = psum.tile([P, P], BF16, name="mkT_ps", tag="tr",
                                   bufs=2)
                nc.tensor.transpose(
                    out=mkT_ps[:], in_=mk_sb[:, ch, :], identity=id128b[:]
                )
                nc.scalar.activation(mkT_sb[:, b, ch, :], mkT_ps[:], ACT.Identity)
            nc.tensor.matmul(
                out=ed_ps[:, ch, :],
                lhsT=mkT_sb[:, b, ch, :],
                rhs=henm_cur[:, b, H:H2],
                start=True,
                stop=True,
            )
        e_sb = sb.tile([P, CH, H], F32, name="e_sb", tag="e_sb", bufs=3)
        nc.vector.tensor_add(out=e_sb[:], in0=gt[:, :, D:ROW], in1=ed_ps[:])
        e2_sb = sb.tile([P, CH, H], F32, name="e2_sb", tag="e2_sb", bufs=3)
        nc.vector.scalar_tensor_tensor(
            out=e2_sb[:], in0=e_sb[:], scalar=NEG, in1=e_sb[:],
            op0=ALU.mult, op1=ALU.max,
        )
        ee_sb = sb.tile([P, CH, H], BF16, name="ee_sb", tag="ee_sb", bufs=3)
        nc.scalar.activation(ee_sb[:], e2_sb[:], ACT.Exp)
        gs = sb.tile([P, CH, D], BF16, name="gs", tag="gs", bufs=2)
        nc.vector.tensor_tensor(
            out=gs[:].rearrange("p c (h f) -> p c h f", h=H),
            in0=gt[:, :, 0:D].rearrange("p c (h f) -> p c h f", h=H),
            in1=ee_sb[:].unsqueeze(3).to_broadcast([P, CH, H, D // H]),
            op=ALU.mult,
        )
        return mk_sb, ee_sb, gs

    def back(l, b, fr):
        mk_sb, ee_sb, gs = fr
        bs = slice(b * P, (b + 1) * P)
        out_ps = psum.tile([P, D], F32, name="out_ps", tag="big", bufs=2)
        den_ps = psum.tile([P, H], F32, name="den_ps", tag="den", bufs=1)
        for ch in range(CH):
            nc.tensor.matmul(
                out=out_ps[:], lhsT=mk_sb[:, ch, :], rhs=gs[:, ch, :],
                start=(ch == 0), stop=(ch == CH - 1),
            )
            nc.tensor.matmul(
                out=den_ps[:], lhsT=mk_sb[:, ch, :], rhs=ee_sb[:, ch, :],
                start=(ch == 0), stop=(ch == CH - 1),
            )
        # ----- epilogue: alpha-normalize, LN (bn_stats + Newton rsqrt), relu
        rec_sb = sb.tile([P, H], F32, name="rec_sb", tag="rec_sb", bufs=2)
        nc.vector.tensor_scalar_add(out=rec_sb[:], in0=den_ps[:], scalar1=1e-16)
        nc.vector.reciprocal(out=rec_sb[:], in_=rec_sb[:])
        y_sb = sb.tile([P, D], F32, name="y_sb", tag="y_sb", bufs=2)
        nc.vector.tensor_tensor(
            out=y_sb[:].rearrange("p (h c) -> p h c", h=H),
            in0=out_ps[:].rearrange("p (h c) -> p h c", h=H),
            in1=rec_sb[:].unsqueeze(2).to_broadcast([P, H, D // H]),
            op=ALU.mult,
        )
        if b_rep is not None:
            nc.vector.tensor_add(out=y_sb[:], in0=y_sb[:], in1=b_rep[l][:])
        st_sb = sb.tile([P, 6], F32, name="st_sb", tag="st_sb", bufs=2)
        nc.vector.bn_stats(out=st_sb[:], in_=y_sb[:])
        mv_sb = sb.tile([P, 2], F32, name="mv_sb", tag="mv_sb", bufs=2)
        nc.vector.bn_aggr(out=mv_sb[:], in_=st_sb[:])
        # rstd = 1/sqrt(var+eps) via magic-seed Newton (all on DVE, fp32)
        nw = sb.tile([P, 6], F32, name="nw", tag="nw", bufs=2)
        ve = nw[:, 0:1]
        nc.vector.tensor_scalar_add(out=ve, in0=mv_sb[:, 1:2], scalar1=float(EPS))
        nc.vector.tensor_scalar(
            out=nw[:, 1:2].bitcast(I32), in0=ve.bitcast(I32),
            scalar1=1, scalar2=None, op0=ALU.arith_shift_right,
        )
        nc.vector.tensor_scalar(
            out=nw[:, 2:3].bitcast(I32), in0=nw[:, 1:2].bitcast(I32),
            scalar1=-1, scalar2=MAGIC, op0=ALU.mult, op1=ALU.add,
        )
        r = nw[:, 2:3]
        for it in range(2):
            a = nw[:, 3:4]
            nc.vector.tensor_mul(out=a, in0=r, in1=r)
            nc.vector.tensor_mul(out=a, in0=ve, in1=a)
            nc.vector.tensor_scalar(
                out=a, in0=a, scalar1=-0.5, scalar2=1.5,
                op0=ALU.mult, op1=ALU.add,
            )
            rn = nw[:, 4 + it : 5 + it]
            nc.vector.tensor_mul(out=rn, in0=r, in1=a)
            r = rn
        nmm2 = sb.tile([P, 1], F32, name="nmm2", tag="nmm2", bufs=2)
        nc.vector.tensor_scalar_mul(out=nmm2[:], in0=mv_sb[:, 0:1], scalar1=-1.0)
        nc.vector.tensor_mul(out=nmm2[:], in0=nmm2[:], in1=r)
        xnb = sb.tile([P, D], BF16, name="xnb", tag="xnb", bufs=2)
        if g_rep is None and be_rep is None:
            # xnb = relu(y * rstd - mean * rstd) in one scalar-engine pass
            nc.scalar.activation(
                xnb[:], y_sb[:], ACT.Relu,
                bias=nmm2[:, 0:1], scale=r,
            )
        else:
            ln_sb = sb.tile([P, D], F32, name="ln_sb", tag="ln_sb", bufs=2)
            nc.vector.scalar_tensor_tensor(
                out=ln_sb[:], in0=y_sb[:], scalar=r,
                in1=nmm2[:].to_broadcast([P, D]), op0=ALU.mult, op1=ALU.add,
            )
            if g_rep is not None:
                nc.vector.tensor_mul(out=ln_sb[:], in0=ln_sb[:], in1=g_rep[l][:])
            if be_rep is not None:
                nc.vector.tensor_add(out=ln_sb[:], in0=ln_sb[:], in1=be_rep[l][:])
            nc.vector.tensor_scalar_max(out=xnb[:], in0=ln_sb[:], scalar1=0.0)

        if l < L - 1:
            for k in range(KD):
                t_ps = psum.tile([P, P], BF16, name="t_ps", tag="tr", bufs=2)
                nc.tensor.transpose(
                    out=t_ps[:], in_=xnb[:, k * P : (k + 1) * P],
                    identity=id128b[:],
                )
                nc.scalar.activation(xT_sb[k][:, bs], t_ps[:], ACT.Identity)
            gemm_he_stage(l + 1, b)
        else:
            nc.tensor.matmul(
                out=pool_ps[:], lhsT=pm_sb[:, b, :], rhs=xnb[:],
                start=(b == 0), stop=(b == NB - 1),
            )

    for l in range(L):
        fr = None
        for b in range(NB):
            f_new = front(l, b)
            if fr is not None:
                back(l, b - 1, fr)
            fr = f_new
        back(l, NB - 1, fr)
        if l < L - 1:
            allgather(l + 1)

    # ---------------- tail: per-core FC on the partial pool, then a small
    # AllReduce of the [G, OUT] logits (FC is linear in the pooled sum)
    pool_sb = res.tile([G, D], F32, name="pool_sb")
    nc.vector.tensor_copy(out=pool_sb[:], in_=pool_ps[:])
    idG = res.tile([G, G], F32, name="idG")
    make_identity(nc, idG[:])
    pT_sb = res.tile([P, KD, G], F32, name="pT_sb")
    for k in range(KD):
        t2_ps = psum.tile([P, G], F32, name="t2_ps", tag="big", bufs=2)
        nc.tensor.transpose(
            out=t2_ps[:], in_=pool_sb[:, k * P : (k + 1) * P], identity=idG[:]
        )
        nc.vector.tensor_copy(out=pT_sb[:, k, :], in_=t2_ps[:])
    fcw_sb = res.tile([P, KD, OUT], F32, name="fcw_sb")
    nc.sync.dma_start(
        out=fcw_sb[:], in_=I["fc_W"][:].rearrange("k p o -> p k o")
    )
    fc_ps = psum.tile([G, OUT], F32, name="fc_ps", tag="big", bufs=2)
    for k in range(KD):
        nc.tensor.matmul(
            out=fc_ps[:], lhsT=pT_sb[:, k, :], rhs=fcw_sb[:, k, :],
            start=(k == 0), stop=(k == KD - 1),
        )
    fcp_sb = res.tile([G, OUT], F32, name="fcp_sb")
    nc.vector.tensor_copy(out=fcp_sb[:], in_=fc_ps[:])
    nc.sync.dma_start(out=ar_in[:], in_=fcp_sb[:])
    nc.gpsimd.collective_compute(
        "AllReduce", ALU.add, replica_groups=rg,
        ins=[ar_in.opt()], outs=[ar_out.opt()],
    )
    pf_sb = res.tile([G, OUT], F32, name="pf_sb")
    nc.sync.dma_start(out=pf_sb[:], in_=ar_out[:])
    ic_sb = res.tile([G, 1], F32, name="ic_sb")
    nc.sync.dma_start(out=ic_sb[:], in_=I["invcnt"][:])
    o_sb = res.tile([G, OUT], F32, name="o_sb")
    nc.vector.tensor_tensor(
        out=o_sb[:], in0=pf_sb[:],
        in1=ic_sb[:].to_broadcast([G, OUT]), op=ALU.mult,
    )
    if not meta["skip_fcb"]:
        fcb_rep = res.tile([P, OUT], F32, name="fcb_rep")
        nc.sync.dma_start(out=fcb_rep[:], in_=I["fcb_rep"][:])
        nc.vector.tensor_add(out=o_sb[:], in0=o_sb[:], in1=fcb_rep[0:G, :])
    nc.sync.dma_start(out=out_ap[:], in_=o_sb[:])
    ctx.close()


# --------------------------------------------------------------------------
# Entry point
# --------------------------------------------------------------------------


def kernel(**inputs):
    global LAST_RESULTS
    cfg = _full_cfg()
    in_maps, meta = _prep(inputs, cfg)

    nc = bacc.Bacc(
        "TRN2",
        target_bir_lowering=False,
        debug=False,
        enable_asserts=False,
        num_devices=cfg["n_cores"],
    )
    I = {}
    for name, arr in in_maps[0].items():
        I[name] = nc.dram_tensor(
            name, arr.shape, mybir.dt.from_np(arr.dtype), kind="ExternalInput"
        ).ap()
    out_ap = nc.dram_tensor(
        "out", (cfg["G"], cfg["OUT"]), F32, kind="ExternalOutput"
    ).ap()

    with tile.TileContext(nc) as tc:
        build(tc, cfg, meta, I, out_ap)
    nc.compile()

    trace = bool(int(os.environ.get("GAT_TRACE", "0")))
    res = run_bass_kernel_spmd(
        nc,
        in_maps,
        core_ids=list(range(cfg["n_cores"])),
        trace=trace,
    )
    LAST_RESULTS = res
    return np.asarray(res.results[0]["out"])
